# revision 13
# baseline (speedup 1.0000x reference)
"""2-layer GAT (PyG GATConv semantics) on 8 Trainium2 NeuronCores via Bass/Tile.

Contract: kernel(**inputs) takes the FULL inputs of reference.setup_inputs()
and returns the FULL [16, 4096, 128] float32 output.

v3 strategy (per-dst-row slots + batched dma_gather), redesigned around the
measured bottleneck of v2 (Pool engine 92% busy: ~1us fixed SWDGE cost per
128-row indirect DMA; 123us per AllGather slice):

- Node->(core, block, partition) assignment is OURS to choose. Cores get
  nodes round-robin by total in-degree; within a core, nodes are lex-sorted
  by (n_lo, n_hi) in-edge counts so each 128-node block has near-uniform
  per-partition slot counts. Every edge becomes slot (p, s) where p = its
  dst's partition: aggregation is a plain row-wise masked reduce on DVE
  (no one-hot routing matrix, no PE transposes, no per-edge a_dst gathers).
- Values are fetched with dma_gather (HW limit: 1024 idxs/instruction, int16
  idx) from 256B-row tables. The int16 range forces a lo/hi table split:
  each block's slots are grouped into a lo section (src phys < 32768) and a
  hi section, gathered by instructions based at the respective table half.
  Pad slots gather row 0 and are masked (w=0). Groups of GRP=4 blocks share
  section heights so gathers batch and DVE ops cover whole groups.
- t1 row = [h1 (64, c-major) | asrc1 (8)] + pad to 128 cols. a_dst1 of OWN
  nodes is host-precomputed (adw1 input) - no cross-core offsets anywhere,
  the program is truly SPMD; all per-core variation is input data.
- Layer-2 linearity: out2 = (sum alpha * elu1[src]) @ W2 + b2, so t2 rows
  carry only [v=elu1 (64) | asrc2' (1)] and W2 is applied per-block AFTER
  aggregation (1 PE transpose + 1 matmul per 128 nodes). a_dst2' of own
  nodes never touches DRAM: phase B parks it in an SBUF tile for phase C.
- t2s shards ([NSH, 65] packed) AllGather into the strided [N, 128] t2
  table in NSLICE slices overlapped with phase-B compute.
"""

import os
import sys

import numpy as np

if "/opt/trn_rl_repo" not in sys.path:
    sys.path.insert(0, "/opt/trn_rl_repo")

import concourse.bass as bass
import concourse.bacc as bacc
import concourse.mybir as mybir
import concourse.tile as tile

F32 = mybir.dt.float32
BF16 = mybir.dt.bfloat16
I16 = mybir.dt.int16
AOP = mybir.AluOpType
ACT = mybir.ActivationFunctionType
AXL = mybir.AxisListType

NEG_SLOPE = 0.2
NCORES = 8
BLK = 128
GRP = 4          # blocks per group
T1C = 72         # t1 used cols: [0:64] h1 (c-major), [64:72] asrc1
T2C = 65         # t2s cols: [0:64] v=elu1, [64] asrc2'
A_OFF = 64       # asrc col offset within a gathered row (both layers)
ROWC = 128       # table row stride in elems (256B = dma_gather elem)
MAXIDX = 1024    # HW limit per dma_gather instruction
CHST = MAXIDX // 128  # stripes per gather chunk


class Cfg:
    def __init__(self, n_nodes, d_in, h1, c1, d2, s_lo, s_hi, nslice):
        self.N = n_nodes
        self.D = d_in
        self.H1 = h1
        self.C1 = c1
        self.D1 = h1 * c1
        self.D2 = d2
        self.NSH = n_nodes // NCORES
        self.NBLK = self.NSH // BLK
        self.NGRP = self.NBLK // GRP
        self.NSLICE = nslice
        self.S_LO = [int(v) for v in s_lo]   # per group: stripes/block, lo sec
        self.S_HI = [int(v) for v in s_hi]
        assert len(self.S_LO) == self.NGRP and len(self.S_HI) == self.NGRP
        assert self.NGRP % nslice == 0
        # chunk tables: per group, list of (sec, stripe0_in_sec, nstripes, col)
        self.chunks = []
        self.grp_off = []        # stripe offset of each group in mask array
        self.tot_stripes = 0
        col = 0
        for g in range(self.NGRP):
            self.grp_off.append(self.tot_stripes)
            ch = []
            for sec, ns in ((0, GRP * self.S_LO[g]), (1, GRP * self.S_HI[g])):
                t0 = 0
                while t0 < ns:
                    k = min(CHST, ns - t0)
                    ch.append((sec, t0, k, col))
                    col += k * 8  # k*128 idxs / 16 per col
                    t0 += k
            self.chunks.append(ch)
            self.tot_stripes += GRP * (self.S_LO[g] + self.S_HI[g])
        self.idx_cols = col


def _ap(t_ap, off, dims):
    """Raw AP view of a tile slice: partition dim kept, free dims replaced.
    `off` in elements, `dims` = [[stride, size], ...]."""
    a = [list(p) for p in t_ap.ap]
    return bass.AP(t_ap.tensor, t_ap.offset + off, [a[0]] + dims)


# ---------------------------------------------------------------------------
# device program
# ---------------------------------------------------------------------------
def build_program(cfg, phases="abc"):
    N, D, D1, D2 = cfg.N, cfg.D, cfg.D1, cfg.D2
    NSH = cfg.NSH

    nc = bacc.Bacc("TRN2", target_bir_lowering=False, debug=False,
                   num_devices=NCORES, num_swdge_queues=4)

    xt = nc.dram_tensor("xt", [D, N], BF16, kind="ExternalInput")
    wpack1 = nc.dram_tensor("wpack1", [D, T1C], BF16, kind="ExternalInput")
    w2t = nc.dram_tensor("w2t", [D1, D2], BF16, kind="ExternalInput")
    w2asr = nc.dram_tensor("w2asr", [128, D1], BF16, kind="ExternalInput")
    w2adr = nc.dram_tensor("w2adr", [128, D1], BF16, kind="ExternalInput")
    b1p = nc.dram_tensor("b1p", [128, D1], F32, kind="ExternalInput")
    b2r = nc.dram_tensor("b2r", [128, D2], F32, kind="ExternalInput")
    identbf = nc.dram_tensor("identbf", [128, 128], BF16, kind="ExternalInput")
    idxw = nc.dram_tensor("idxw", [128, cfg.idx_cols], I16, kind="ExternalInput")
    maskt = nc.dram_tensor("maskt", [128, cfg.tot_stripes], BF16,
                           kind="ExternalInput")
    adw1 = nc.dram_tensor("adw1", [128, cfg.NBLK * cfg.H1], BF16,
                          kind="ExternalInput")
    out = nc.dram_tensor("out", [NSH, D2], F32, kind="ExternalOutput")

    t1 = nc.dram_tensor("t1", [N, ROWC], BF16, kind="Internal")
    t2s = nc.dram_tensor("t2s", [NSH, T2C], BF16, kind="Internal")
    t2tmp = nc.dram_tensor("t2tmp", [N, T2C], BF16, kind="Internal",
                           addr_space="Shared")
    t2 = nc.dram_tensor("t2", [N, ROWC], BF16, kind="Internal")

    with tile.TileContext(nc) as tc:
        with tc.tile_pool(name="const", bufs=1) as cp:
            con = {}
            for name, hndl, dt in [
                ("wpack1", wpack1, BF16), ("w2t", w2t, BF16),
                ("w2asr", w2asr, BF16), ("w2adr", w2adr, BF16),
                ("b1p", b1p, F32), ("b2r", b2r, F32),
                ("identbf", identbf, BF16), ("idxw", idxw, I16),
                ("maskt", maskt, BF16), ("adw1", adw1, BF16),
            ]:
                t = cp.tile(list(hndl.shape), dt, tag=name)
                nc.sync.dma_start(out=t[:], in_=hndl[:])
                con[name] = t
            # adst2' of own nodes, written by phase B, read by phase C
            adw2_t = cp.tile([128, cfg.NBLK], BF16, tag="adw2")
            con["adw2"] = adw2_t

            rep = int(os.environ.get("KREP", "1"))
            for r in range(rep):
                sfx = f"r{r}" if r else ""
                if "a" in phases:
                    _phase_a(nc, tc, cfg, xt, con["wpack1"], t1, sfx)
                if "b" in phases:
                    _edge_phase(nc, tc, cfg, 1, t1, t2s, (t2tmp, t2), con,
                                None, sfx)
                if "c" in phases:
                    _edge_phase(nc, tc, cfg, 2, t2, t2s, None, con, out, sfx)

    nc.compile()
    return nc


def _phase_a(nc, tc, cfg, xt, wpack1_t, t1, sfx=""):
    N = cfg.N
    ntile = N // 128
    GA = 8  # node tiles per outer step
    with (
        tc.tile_pool(name="pa_in" + sfx, bufs=3) as pin,
        tc.tile_pool(name="pa_ps" + sfx, bufs=4, space="PSUM") as pps,
        tc.tile_pool(name="pa_st" + sfx, bufs=3) as pst,
    ):
        for mt in range(ntile // GA):
            xt_t = pin.tile([128, 128 * GA], BF16, tag="xt")
            nc.sync.dma_start(
                out=xt_t[:], in_=xt[:, mt * 128 * GA : (mt + 1) * 128 * GA]
            )
            stg = pst.tile([128, GA * T1C], BF16, tag="stg")
            for half in range(2):
                ps = pps.tile([128, 4 * T1C], F32, tag="ps")
                for s in range(4):
                    st = half * 4 + s
                    nc.tensor.matmul(
                        out=ps[:, s * T1C : (s + 1) * T1C],
                        lhsT=xt_t[:, st * 128 : (st + 1) * 128],
                        rhs=wpack1_t[:], start=True, stop=True,
                    )
                dstv = stg[:, half * 4 * T1C : (half + 1) * 4 * T1C]
                if half == 0:
                    nc.vector.tensor_copy(out=dstv, in_=ps[:])
                else:
                    nc.scalar.copy(out=dstv, in_=ps[:])
            dst_ap = bass.AP(
                t1[:].tensor,
                mt * 128 * GA * ROWC,
                [[ROWC, 128], [128 * ROWC, GA], [1, T1C]],
            )
            nc.sync.dma_start(
                out=dst_ap, in_=stg[:].rearrange("p (s w) -> p s w", w=T1C)
            )


def _edge_phase(nc, tc, cfg, layer, vtab, t2s, t2, con, out, sfx=""):
    """layer 1: gathers from t1, writes t2s rows + sliced AllGather into t2.
    layer 2: gathers from t2, writes out rows."""
    NGRP, NSLICE = cfg.NGRP, cfg.NSLICE
    H1, C1, D1 = cfg.H1, cfg.C1, cfg.D1
    H = H1 if layer == 1 else 1
    GPS = NGRP // NSLICE  # groups per collective slice
    L = f"e{layer}" + sfx
    idxt = con["idxw"]
    maskt = con["maskt"]
    halfrows = cfg.N // 2

    with (
        tc.tile_pool(name=L + "_g", bufs=2) as pg,
        tc.tile_pool(name=L + "_w", bufs=2) as pw,
        tc.tile_pool(name=L + "_e", bufs=2) as pe,
        tc.tile_pool(name=L + "_ps", bufs=2, space="PSUM") as pps,
        tc.tile_pool(name=L + "_ps2", bufs=2, space="PSUM") as pps2,
    ):
        for g in range(NGRP):
            SL, SH = cfg.S_LO[g], cfg.S_HI[g]
            TLO, THI = GRP * SL, GRP * SH
            T = TLO + THI
            goff = cfg.grp_off[g]

            vg = pg.tile([128, T * ROWC], BF16, tag="vg")
            for ci, (sec, t0, k, coloff) in enumerate(cfg.chunks[g]):
                base = TLO if sec else 0
                nidx = k * 128
                nc.gpsimd.dma_gather(
                    out_ap=_ap(vg[:], (base + t0) * ROWC,
                               [[ROWC, k], [1, ROWC]]),
                    in_ap=(vtab[halfrows : 2 * halfrows, :] if sec
                           else vtab[0:halfrows, :]),
                    idxs_ap=idxt[:, coloff : coloff + k * 8],
                    num_idxs=nidx, num_idxs_reg=nidx, elem_size=ROWC,
                    queue_num=int(os.environ.get("KGQ", "0")) and ci % 4,
                )

            # logits lg[p, t*H + h] = asrc[src] + adst[own node p]
            lg = pw.tile([128, T * H], BF16, tag="lg")
            for soff, scnt in ((0, SL), (TLO, SH)):
                if scnt == 0:
                    continue
                if layer == 1:
                    adw_in = _ap(con["adw1"][:], g * GRP * H1,
                                 [[H1, GRP], [0, scnt], [1, H1]])
                else:
                    adw_in = _ap(con["adw2"][:], g * GRP,
                                 [[1, GRP], [0, scnt], [0, 1]])
                nc.vector.tensor_tensor(
                    out=_ap(lg[:], soff * H,
                            [[scnt * H, GRP], [H, scnt], [1, H]]),
                    in0=_ap(vg[:], soff * ROWC + A_OFF,
                            [[scnt * ROWC, GRP], [ROWC, scnt], [1, H]]),
                    in1=adw_in,
                    op=AOP.add,
                )
            lr = pw.tile([128, T * H], BF16, tag="lr")
            nc.vector.scalar_tensor_tensor(
                out=lr[:], in0=lg[:], scalar=NEG_SLOPE, in1=lg[:],
                op0=AOP.mult, op1=AOP.max,
            )
            w_t = pw.tile([128, T * H], BF16, tag="w")
            nc.scalar.activation(out=w_t[:], in_=lr[:], func=ACT.Exp)
            wm = pw.tile([128, T * H], BF16, tag="wm")
            nc.vector.tensor_tensor(
                out=wm[:], in0=w_t[:],
                in1=_ap(maskt[:], goff, [[1, T], [0, H]]),
                op=AOP.mult,
            )

            # weighted values, in place into vg's value cols
            VC = D1 if layer == 1 else 64
            if layer == 1:
                nc.vector.tensor_tensor(
                    out=_ap(vg[:], 0, [[ROWC, T], [H1, C1], [1, H1]]),
                    in0=_ap(vg[:], 0, [[ROWC, T], [H1, C1], [1, H1]]),
                    in1=_ap(wm[:], 0, [[H1, T], [0, C1], [1, H1]]),
                    op=AOP.mult,
                )
            else:
                nc.vector.tensor_tensor(
                    out=_ap(vg[:], 0, [[ROWC, T], [1, 64]]),
                    in0=_ap(vg[:], 0, [[ROWC, T], [1, 64]]),
                    in1=_ap(wm[:], 0, [[1, T], [0, 64]]),
                    op=AOP.mult,
                )

            # per-block reduce over stripes (lo + hi)
            vacc = pe.tile([128, GRP * VC], F32, tag="vacc")
            nc.vector.tensor_reduce(
                out=vacc[:],
                in_=_ap(vg[:], 0, [[SL * ROWC, GRP], [1, VC], [ROWC, SL]]),
                axis=AXL.X, op=AOP.add,
            )
            wsum = pe.tile([128, GRP * H], F32, tag="wsum")
            nc.vector.tensor_reduce(
                out=wsum[:],
                in_=_ap(wm[:], 0, [[SL * H, GRP], [1, H], [H, SL]]),
                axis=AXL.X, op=AOP.add,
            )
            if THI:
                va2 = pe.tile([128, GRP * VC], F32, tag="va2")
                nc.vector.tensor_reduce(
                    out=va2[:],
                    in_=_ap(vg[:], TLO * ROWC,
                            [[SH * ROWC, GRP], [1, VC], [ROWC, SH]]),
                    axis=AXL.X, op=AOP.add,
                )
                nc.vector.tensor_add(out=vacc[:], in0=vacc[:], in1=va2[:])
                ws2 = pe.tile([128, GRP * H], F32, tag="ws2")
                nc.vector.tensor_reduce(
                    out=ws2[:],
                    in_=_ap(wm[:], TLO * H, [[SH * H, GRP], [1, H], [H, SH]]),
                    axis=AXL.X, op=AOP.add,
                )
                nc.vector.tensor_add(out=wsum[:], in0=wsum[:], in1=ws2[:])

            sinv = pe.tile([128, GRP * H], F32, tag="sinv")
            nc.vector.reciprocal(out=sinv[:], in_=wsum[:])

            if layer == 1:
                _l1_epilogue(nc, cfg, g, con, pe, vacc, sinv, t2s)
                if t2 is not None and (g + 1) % GPS == 0:
                    t2tmp_h, t2_h = t2
                    s = (g + 1) // GPS - 1
                    SLN = cfg.NSH // NSLICE
                    nc.gpsimd.collective_compute(
                        "AllGather", AOP.bypass,
                        replica_groups=[list(range(NCORES))],
                        ins=[t2s[s * SLN : (s + 1) * SLN, :]],
                        outs=[t2tmp_h[s * SLN * NCORES :
                                      (s + 1) * SLN * NCORES, :]],
                    )
                    # local restride: [core, SLN, 65] -> t2 rows c*NSH+s*SLN+i
                    nc.sync.dma_start(
                        out=bass.AP(
                            t2_h[:].tensor, s * SLN * ROWC,
                            [[cfg.NSH * ROWC, NCORES], [ROWC, SLN], [1, T2C]],
                        ),
                        in_=bass.AP(
                            t2tmp_h[:].tensor, s * SLN * NCORES * T2C,
                            [[SLN * T2C, NCORES], [T2C, SLN], [1, T2C]],
                        ),
                    )
            else:
                _l2_epilogue(nc, cfg, g, con, pe, pps, pps2, vacc, sinv, out)


def _l1_epilogue(nc, cfg, g, con, pe, vacc, sinv, t2s):
    H1, C1, D1 = cfg.H1, cfg.C1, cfg.D1
    # y = vacc/wsum per head (c-major) + b1
    y = pe.tile([128, GRP * D1], F32, tag="y")
    nc.vector.tensor_tensor(
        out=y[:], in0=vacc[:],
        in1=_ap(sinv[:], 0, [[H1, GRP], [0, C1], [1, H1]]),
        op=AOP.mult,
    )
    nc.vector.tensor_tensor(
        out=y[:], in0=y[:],
        in1=_ap(con["b1p"][:], 0, [[0, GRP], [1, D1]]),
        op=AOP.add,
    )
    tmin = pe.tile([128, GRP * D1], F32, tag="tmin")
    nc.vector.tensor_scalar_min(out=tmin[:], in0=y[:], scalar1=0.0)
    e_t = pe.tile([128, GRP * D1], F32, tag="e")
    nc.scalar.activation(out=e_t[:], in_=tmin[:], func=ACT.Exp)
    helu = pe.tile([128, GRP * D1], F32, tag="helu")
    nc.vector.scalar_tensor_tensor(
        out=helu[:], in0=y[:], scalar=0.0, in1=e_t[:],
        op0=AOP.max, op1=AOP.add,
    )
    vq = pe.tile([128, GRP * D1], BF16, tag="vq")
    nc.vector.tensor_scalar_add(out=vq[:], in0=helu[:], scalar1=-1.0)
    # asrc2' = vq . (W2 @ a2s) ; adst2' = vq . (W2 @ a2d)
    asr = pe.tile([128, GRP], F32, tag="asr")
    adt = pe.tile([128, GRP], F32, tag="adt")
    for wsname, dst in (("w2asr", asr), ("w2adr", adt)):
        tmp = pe.tile([128, GRP * D1], BF16, tag="tmp" + wsname)
        nc.vector.tensor_tensor(
            out=tmp[:], in0=vq[:],
            in1=_ap(con[wsname][:], 0, [[0, GRP], [1, D1]]),
            op=AOP.mult,
        )
        nc.vector.tensor_reduce(
            out=dst[:], in_=_ap(tmp[:], 0, [[D1, GRP], [1, D1]]),
            axis=AXL.X, op=AOP.add,
        )
    nc.vector.tensor_copy(
        out=con["adw2"][:, g * GRP : (g + 1) * GRP], in_=adt[:]
    )
    stg = pe.tile([128, GRP * T2C], BF16, tag="stg")
    nc.vector.tensor_copy(
        out=_ap(stg[:], 0, [[T2C, GRP], [1, D1]]),
        in_=_ap(vq[:], 0, [[D1, GRP], [1, D1]]),
    )
    nc.scalar.copy(
        out=_ap(stg[:], D1, [[T2C, GRP], [1, 1]]),
        in_=_ap(asr[:], 0, [[1, GRP], [1, 1]]),
    )
    nc.sync.dma_start(
        out=bass.AP(
            t2s[:].tensor, g * GRP * 128 * T2C,
            [[T2C, 128], [128 * T2C, GRP], [1, T2C]],
        ),
        in_=stg[:].rearrange("p (b w) -> p b w", w=T2C),
    )


def _l2_epilogue(nc, cfg, g, con, pe, pps, pps2, vacc, sinv, out):
    D2 = cfg.D2
    hm = pe.tile([128, GRP * 64], BF16, tag="hm")
    nc.vector.tensor_tensor(
        out=hm[:], in0=vacc[:],
        in1=_ap(sinv[:], 0, [[1, GRP], [0, 64]]),
        op=AOP.mult,
    )
    for b in range(GRP):
        pt = pps.tile([64, 128], BF16, tag="pt")
        nc.tensor.transpose(
            out=pt[:], in_=hm[:, b * 64 : (b + 1) * 64],
            identity=con["identbf"][:],
        )
        hT = pe.tile([64, 128], BF16, tag="hT")
        if b % 2 == 0:
            nc.vector.tensor_copy(out=hT[:], in_=pt[:])
        else:
            nc.scalar.copy(out=hT[:], in_=pt[:])
        ps2 = pps2.tile([128, D2], F32, tag="ps2")
        nc.tensor.matmul(
            out=ps2[:], lhsT=hT[:], rhs=con["w2t"][:], start=True, stop=True
        )
        o_t = pe.tile([128, D2], F32, tag="o")
        nc.vector.tensor_add(out=o_t[:], in0=ps2[:], in1=con["b2r"][:])
        nc.sync.dma_start(
            out=out[(g * GRP + b) * 128 : (g * GRP + b + 1) * 128, :],
            in_=o_t[:],
        )


# ---------------------------------------------------------------------------
# host-side schedule + glue
# ---------------------------------------------------------------------------
def _schedule(src, dst, N):
    """Returns (node_tab [NC, NSH], S_LO [NGRP], S_HI [NGRP], slot arrays).
    Slot arrays are per-edge: core, chunk position info, mask positions."""
    NSH = N // NCORES
    NBLK = NSH // BLK
    NGRP = NBLK // GRP
    # self-loops as ordinary edges
    srcs = np.concatenate([src, np.arange(N, dtype=np.int64)])
    dsts = np.concatenate([dst, np.arange(N, dtype=np.int64)])
    deg = np.bincount(dsts, minlength=N)

    # halves balanced by degree; nlo/nhi depend only on half membership
    order = np.argsort(-deg, kind="stable")
    half_of = np.empty(N, dtype=np.int64)
    half_of[order] = np.arange(N) % 2
    is_lo = half_of == 0

    hi_flag = (~is_lo[srcs]).astype(np.int64)
    nlo = np.bincount(dsts[hi_flag == 0], minlength=N)
    nhi = deg - nlo

    # per half: ONE global lex sort by (nlo, nhi); consecutive runs of
    # 4*GRP*BLK nodes form group g across the half's 4 cores (dealt
    # round-robin), so every core's group-g block set is statistically
    # identical -> the global per-group section maxima stay tight.
    HC = NCORES // 2
    RUN = HC * GRP * BLK
    NGRP = NSH // (GRP * BLK)
    node_tab = np.empty((NCORES, NSH), dtype=np.int64)
    for h in range(2):
        nodes = np.where(half_of == h)[0]
        # quantile grid: 4 nlo bands (4 runs each), nhi-sorted within band,
        # so each run is tight in BOTH nlo and nhi
        nl = nlo[nodes]
        o1 = np.argsort(nl, kind="stable")
        qlo = np.empty(len(nodes), dtype=np.int64)
        qlo[o1] = np.arange(len(nodes)) // (4 * RUN)
        o = np.lexsort((nhi[nodes], qlo))
        runs = nodes[o].reshape(NGRP, GRP * BLK, HC)
        node_tab[h * HC : (h + 1) * HC] = runs.transpose(2, 0, 1).reshape(
            HC, NGRP * GRP * BLK)
    phys = np.empty(N, dtype=np.int64)
    phys[node_tab.reshape(-1)] = np.arange(N)

    # per-group section heights (global max over cores)
    nlo_b = nlo[node_tab].reshape(NCORES, NGRP, GRP * BLK)
    nhi_b = nhi[node_tab].reshape(NCORES, NGRP, GRP * BLK)
    S_LO = np.maximum(nlo_b.max(axis=(0, 2)), 1)
    S_HI = np.maximum(nhi_b.max(axis=(0, 2)), 1)

    # per-edge rank within (dst, sec)
    eo = np.lexsort((hi_flag, dsts))
    sd = dsts[eo]
    sh = hi_flag[eo]
    ss = srcs[eo]
    key_change = np.ones(len(sd), dtype=bool)
    key_change[1:] = (sd[1:] != sd[:-1]) | (sh[1:] != sh[:-1])
    gidx = np.flatnonzero(key_change)
    grp_id = np.cumsum(key_change) - 1
    rank = np.arange(len(sd)) - gidx[grp_id]

    return (node_tab, phys, S_LO, S_HI, sd, sh, ss, rank)


def prepare(x, seq, edges, W1, att_src1, att_dst1, b1, W2, att_src2,
            att_dst2, b2, nslice=4):
    import ml_dtypes

    bf = ml_dtypes.bfloat16
    nb, ncn, d = x.shape
    N = nb * ncn
    H1, C1 = att_src1.shape
    D1 = H1 * C1
    D2 = W2.shape[1]
    NSH = N // NCORES
    NBLK = NSH // BLK

    xf = (np.asarray(x, np.float32).reshape(N, d)
          * np.asarray(seq, np.float32).reshape(N, 1))
    src = np.asarray(edges[0], np.int64)
    dst = np.asarray(edges[1], np.int64)
    node_tab, phys, S_LO, S_HI, sd, sh, ss, rank = _schedule(src, dst, N)
    cfg = Cfg(N, d, H1, C1, D2, S_LO, S_HI, nslice)

    # ---- weights / consts -------------------------------------------------
    new2old = np.empty(D1, dtype=np.int64)
    for c in range(C1):
        for h in range(H1):
            new2old[c * H1 + h] = h * C1 + c
    w1 = np.asarray(W1, np.float32)
    wsrc = np.einsum("khc,hc->kh", w1.reshape(d, H1, C1),
                     np.asarray(att_src1, np.float32))
    wdst = np.einsum("khc,hc->kh", w1.reshape(d, H1, C1),
                     np.asarray(att_dst1, np.float32))
    wpack1 = np.concatenate([w1[:, new2old], wsrc], axis=1).astype(bf)

    w2p = np.asarray(W2, np.float32)[new2old, :]
    a2s = np.asarray(att_src2, np.float32).reshape(-1)
    a2d = np.asarray(att_dst2, np.float32).reshape(-1)
    w2as = w2p @ a2s    # [D1] c-major
    w2ad = w2p @ a2d
    w2asr = np.tile(w2as[None, :], (128, 1)).astype(bf)
    w2adr = np.tile(w2ad[None, :], (128, 1)).astype(bf)
    b1p = np.tile(np.asarray(b1, np.float32)[new2old][None, :],
                  (128, 1)).astype(np.float32)
    b2r = np.tile(np.asarray(b2, np.float32)[None, :], (128, 1)).astype(
        np.float32)
    identbf = np.eye(128, dtype=np.float32).astype(bf)

    # xt: global phys order (same for every core)
    xt = np.ascontiguousarray(xf[node_tab.reshape(-1)].T).astype(bf)

    # adw1: a_dst1 of own nodes, [128, NBLK*H1] per core
    ad_all = (xf @ wdst).astype(np.float32)          # [N, H1]
    adw1_all = ad_all[node_tab].reshape(NCORES, NBLK, BLK, H1)
    adw1_all = adw1_all.transpose(0, 2, 1, 3).reshape(
        NCORES, BLK, NBLK * H1).astype(bf)

    # ---- slot placement (vectorized) -------------------------------------
    # per-edge destination placement
    pd = phys[sd]                       # phys row of dst
    e_core = pd // NSH
    loc = pd % NSH
    e_blk = loc // BLK                  # block in core
    e_p = loc % BLK                     # partition
    e_g = e_blk // GRP
    e_b = e_blk % GRP
    slo_g = S_LO[e_g]
    shi_g = S_HI[e_g]
    # stripe within the group's section-stripe space
    ts = np.where(sh == 0, e_b * slo_g + rank, e_b * shi_g + rank)
    # chunk col offsets: build lookup per (group, sec, chunk_index)
    chcol = {}
    for g in range(cfg.NGRP):
        for (sec, t0, k, col) in cfg.chunks[g]:
            chcol[(g, sec, t0 // CHST)] = col
    max_ch = max(
        max((t0 // CHST) for (sec, t0, k, col) in cfg.chunks[g]
            if True) for g in range(cfg.NGRP)) + 1
    col_lut = np.full((cfg.NGRP, 2, max_ch + 1), -1, dtype=np.int64)
    for (g, sec, ci), col in chcol.items():
        col_lut[g, sec, ci] = col
    ci = ts // CHST
    tin = ts % CHST
    pos = tin * 128 + e_p               # position within chunk
    basecol = col_lut[e_g, sh, ci]
    assert (basecol >= 0).all()
    idx_col = basecol + pos // 16
    idx_row = pos % 16
    idx_val = np.where(sh == 0, phys[ss], phys[ss] - N // 2).astype(np.int16)
    # mask position: group stripe offset + section offset + ts
    grp_off = np.asarray(cfg.grp_off, dtype=np.int64)
    sec_off = np.where(sh == 0, 0, GRP * slo_g)
    mask_col = grp_off[e_g] + sec_off + ts

    idxw = np.zeros((NCORES, 128, cfg.idx_cols), dtype=np.int16)
    maskv = np.zeros((NCORES, 128, cfg.tot_stripes), dtype=np.float32)
    for c in range(NCORES):
        m = e_core == c
        # idx wrapped layout: row pos%16, replicated over 8 row-groups
        rows = idx_row[m]
        cols = idx_col[m]
        vals = idx_val[m]
        for repk in range(8):
            idxw[c][rows + 16 * repk, cols] = vals
        maskv[c][e_p[m], mask_col[m]] = 1.0
    maskv = maskv.astype(bf)

    in_maps = []
    for c in range(NCORES):
        in_maps.append({
            "xt": xt, "wpack1": wpack1, "w2t": w2p.astype(bf),
            "w2asr": w2asr, "w2adr": w2adr, "b1p": b1p, "b2r": b2r,
            "identbf": identbf, "idxw": idxw[c], "maskt": maskv[c],
            "adw1": adw1_all[c],
        })
    return cfg, node_tab, in_maps


_CACHE = {}
LAST_RESULT = None


def kernel(**inputs) -> np.ndarray:
    from concourse.bass_utils import run_bass_kernel_spmd

    global LAST_RESULT
    x = np.asarray(inputs["x"])
    nb, ncn, d = x.shape
    nslice = int(os.environ.get("KNSLICE", "4"))
    cfg, node_tab, in_maps = prepare(**{k: inputs[k] for k in (
        "x", "seq", "edges", "W1", "att_src1", "att_dst1", "b1",
        "W2", "att_src2", "att_dst2", "b2")}, nslice=nslice)

    phases = os.environ.get("KPHASES", "abc")
    key = (cfg.N, cfg.D, cfg.H1, cfg.C1, cfg.D2, tuple(cfg.S_LO),
           tuple(cfg.S_HI), cfg.NSLICE, phases)
    if key not in _CACHE:
        _CACHE.clear()
        _CACHE[key] = build_program(cfg, phases=phases)
    nc = _CACHE[key]

    res = run_bass_kernel_spmd(nc, in_maps, core_ids=list(range(NCORES)),
                               trace=False)
    LAST_RESULT = res
    shards = np.concatenate(
        [res.results[c]["out"] for c in range(NCORES)], axis=0)
    full = np.empty((cfg.N, cfg.D2), dtype=np.float32)
    full[node_tab.reshape(-1)] = shards
    return full.reshape(nb, ncn, d).astype(np.float32)


# revision 16
# speedup vs baseline: 1.8187x; 1.8187x over previous
"""2-layer GAT (PyG GATConv semantics) on 8 Trainium2 NeuronCores via Bass/Tile.

Contract: kernel(**inputs) takes the FULL inputs of reference.setup_inputs()
and returns the FULL [16, 4096, 128] float32 output.

v3 strategy (per-dst-row slots + batched dma_gather), redesigned around the
measured bottleneck of v2 (Pool engine 92% busy: ~1us fixed SWDGE cost per
128-row indirect DMA; 123us per AllGather slice):

- Node->(core, block, partition) assignment is OURS to choose. Cores get
  nodes round-robin by total in-degree; within a core, nodes are lex-sorted
  by (n_lo, n_hi) in-edge counts so each 128-node block has near-uniform
  per-partition slot counts. Every edge becomes slot (p, s) where p = its
  dst's partition: aggregation is a plain row-wise masked reduce on DVE
  (no one-hot routing matrix, no PE transposes, no per-edge a_dst gathers).
- Values are fetched with dma_gather (HW limit: 1024 idxs/instruction, int16
  idx) from 256B-row tables. The int16 range forces a lo/hi table split:
  each block's slots are grouped into a lo section (src phys < 32768) and a
  hi section, gathered by instructions based at the respective table half.
  Pad slots gather row 0 and are masked (w=0). Groups of GRP=4 blocks share
  section heights so gathers batch and DVE ops cover whole groups.
- t1 row = [h1 (64, c-major) | asrc1 (8)] + pad to 128 cols. a_dst1 of OWN
  nodes is host-precomputed (adw1 input) - no cross-core offsets anywhere,
  the program is truly SPMD; all per-core variation is input data.
- Layer-2 linearity: out2 = (sum alpha * elu1[src]) @ W2 + b2, so t2 rows
  carry only [v=elu1 (64) | asrc2' (1)] and W2 is applied per-block AFTER
  aggregation (1 PE transpose + 1 matmul per 128 nodes). a_dst2' of own
  nodes never touches DRAM: phase B parks it in an SBUF tile for phase C.
- t2s shards ([NSH, 65] packed) AllGather into the strided [N, 128] t2
  table in NSLICE slices overlapped with phase-B compute.
"""

import os
import sys

import numpy as np

if "/opt/trn_rl_repo" not in sys.path:
    sys.path.insert(0, "/opt/trn_rl_repo")

import concourse.bass as bass
import concourse.bacc as bacc
import concourse.mybir as mybir
import concourse.tile as tile

F32 = mybir.dt.float32
BF16 = mybir.dt.bfloat16
I16 = mybir.dt.int16
AOP = mybir.AluOpType
ACT = mybir.ActivationFunctionType
AXL = mybir.AxisListType

NEG_SLOPE = 0.2
NCORES = 8
BLK = 128
GRP = 4          # blocks per group
T1C = 72         # t1 used cols: [0:64] h1 (c-major), [64:72] asrc1
T2C = 65         # t2s cols: [0:64] v=elu1, [64] asrc2'
A_OFF = 64       # asrc col offset within a gathered row (both layers)
ROWC = 128       # table row stride in elems (256B = dma_gather elem)
MAXIDX = 1024    # HW limit per dma_gather instruction
CHST = MAXIDX // 128  # stripes per gather chunk


class Cfg:
    def __init__(self, n_nodes, d_in, h1, c1, d2, s_lo, s_hi, nslice):
        self.N = n_nodes
        self.D = d_in
        self.H1 = h1
        self.C1 = c1
        self.D1 = h1 * c1
        self.D2 = d2
        self.NSH = n_nodes // NCORES
        self.NBLK = self.NSH // BLK
        self.NGRP = self.NBLK // GRP
        self.NSLICE = nslice
        self.S_LO = [int(v) for v in s_lo]   # per group: stripes/block, lo sec
        self.S_HI = [int(v) for v in s_hi]
        assert len(self.S_LO) == self.NGRP and len(self.S_HI) == self.NGRP
        assert self.NGRP % nslice == 0
        # chunk tables: per group, list of (sec, stripe0_in_sec, nstripes, col)
        self.chunks = []
        self.grp_off = []        # stripe offset of each group in mask array
        self.tot_stripes = 0
        col = 0
        for g in range(self.NGRP):
            self.grp_off.append(self.tot_stripes)
            ch = []
            for sec, ns in ((0, GRP * self.S_LO[g]), (1, GRP * self.S_HI[g])):
                t0 = 0
                while t0 < ns:
                    k = min(CHST, ns - t0)
                    ch.append((sec, t0, k, col))
                    col += k * 8  # k*128 idxs / 16 per col
                    t0 += k
            self.chunks.append(ch)
            self.tot_stripes += GRP * (self.S_LO[g] + self.S_HI[g])
        self.idx_cols = col


def _ap(t_ap, off, dims):
    """Raw AP view of a tile slice: partition dim kept, free dims replaced.
    `off` in elements, `dims` = [[stride, size], ...]."""
    a = [list(p) for p in t_ap.ap]
    return bass.AP(t_ap.tensor, t_ap.offset + off, [a[0]] + dims)


# ---------------------------------------------------------------------------
# device program
# ---------------------------------------------------------------------------
def build_program(cfg, phases="abc"):
    N, D, D1, D2 = cfg.N, cfg.D, cfg.D1, cfg.D2
    NSH = cfg.NSH

    nc = bacc.Bacc("TRN2", target_bir_lowering=False, debug=False,
                   num_devices=NCORES, num_swdge_queues=4)

    xt = nc.dram_tensor("xt", [D, N], BF16, kind="ExternalInput")
    wpack1 = nc.dram_tensor("wpack1", [D, T1C], BF16, kind="ExternalInput")
    w2t = nc.dram_tensor("w2t", [D1, D2], BF16, kind="ExternalInput")
    w2asr = nc.dram_tensor("w2asr", [128, D1], BF16, kind="ExternalInput")
    w2adr = nc.dram_tensor("w2adr", [128, D1], BF16, kind="ExternalInput")
    b1p = nc.dram_tensor("b1p", [128, D1], F32, kind="ExternalInput")
    b2r = nc.dram_tensor("b2r", [128, D2], F32, kind="ExternalInput")
    identbf = nc.dram_tensor("identbf", [128, 128], BF16, kind="ExternalInput")
    idxw = nc.dram_tensor("idxw", [128, cfg.idx_cols], I16, kind="ExternalInput")
    maskt = nc.dram_tensor("maskt", [128, cfg.tot_stripes], BF16,
                           kind="ExternalInput")
    adw1 = nc.dram_tensor("adw1", [128, cfg.NBLK * cfg.H1], BF16,
                          kind="ExternalInput")
    out = nc.dram_tensor("out", [NSH, D2], F32, kind="ExternalOutput")

    t1 = nc.dram_tensor("t1", [N, ROWC], BF16, kind="Internal")
    t2s = nc.dram_tensor("t2s", [NSH, T2C], BF16, kind="Internal")
    t2tmp = nc.dram_tensor("t2tmp", [N, T2C], BF16, kind="Internal",
                           addr_space="Shared")
    t2 = nc.dram_tensor("t2", [N, ROWC], BF16, kind="Internal")

    with tile.TileContext(nc) as tc:
        with tc.tile_pool(name="const", bufs=1) as cp:
            con = {}
            for name, hndl, dt in [
                ("wpack1", wpack1, BF16), ("w2t", w2t, BF16),
                ("w2asr", w2asr, BF16), ("w2adr", w2adr, BF16),
                ("b1p", b1p, F32), ("b2r", b2r, F32),
                ("identbf", identbf, BF16), ("idxw", idxw, I16),
                ("maskt", maskt, BF16), ("adw1", adw1, BF16),
            ]:
                t = cp.tile(list(hndl.shape), dt, tag=name)
                nc.sync.dma_start(out=t[:], in_=hndl[:])
                con[name] = t
            # adst2' of own nodes, written by phase B, read by phase C
            adw2_t = cp.tile([128, cfg.NBLK], BF16, tag="adw2")
            con["adw2"] = adw2_t

            rep = int(os.environ.get("KREP", "1"))
            for r in range(rep):
                sfx = f"r{r}" if r else ""
                if "a" in phases:
                    _phase_a(nc, tc, cfg, xt, con["wpack1"], t1, sfx)
                if "b" in phases:
                    _edge_phase(nc, tc, cfg, 1, t1, t2s, (t2tmp, t2), con,
                                None, sfx)
                if "c" in phases:
                    _edge_phase(nc, tc, cfg, 2, t2, t2s, None, con, out, sfx)

    nc.compile()
    return nc


def _phase_a(nc, tc, cfg, xt, wpack1_t, t1, sfx=""):
    N = cfg.N
    ntile = N // 128
    GA = 8  # node tiles per outer step
    with (
        tc.tile_pool(name="pa_in" + sfx, bufs=3) as pin,
        tc.tile_pool(name="pa_ps" + sfx, bufs=4, space="PSUM") as pps,
        tc.tile_pool(name="pa_st" + sfx, bufs=3) as pst,
    ):
        for mt in range(ntile // GA):
            xt_t = pin.tile([128, 128 * GA], BF16, tag="xt")
            nc.sync.dma_start(
                out=xt_t[:], in_=xt[:, mt * 128 * GA : (mt + 1) * 128 * GA]
            )
            stg = pst.tile([128, GA * T1C], BF16, tag="stg")
            for half in range(2):
                ps = pps.tile([128, 4 * T1C], F32, tag="ps")
                for s in range(4):
                    st = half * 4 + s
                    nc.tensor.matmul(
                        out=ps[:, s * T1C : (s + 1) * T1C],
                        lhsT=xt_t[:, st * 128 : (st + 1) * 128],
                        rhs=wpack1_t[:], start=True, stop=True,
                    )
                dstv = stg[:, half * 4 * T1C : (half + 1) * 4 * T1C]
                if half == 0:
                    nc.vector.tensor_copy(out=dstv, in_=ps[:])
                else:
                    nc.scalar.copy(out=dstv, in_=ps[:])
            dst_ap = bass.AP(
                t1[:].tensor,
                mt * 128 * GA * ROWC,
                [[ROWC, 128], [128 * ROWC, GA], [1, T1C]],
            )
            nc.sync.dma_start(
                out=dst_ap, in_=stg[:].rearrange("p (s w) -> p s w", w=T1C)
            )


def _edge_phase(nc, tc, cfg, layer, vtab, t2s, t2, con, out, sfx=""):
    """layer 1: gathers from t1, writes t2s rows + sliced AllGather into t2.
    layer 2: gathers from t2, writes out rows."""
    NGRP, NSLICE = cfg.NGRP, cfg.NSLICE
    H1, C1, D1 = cfg.H1, cfg.C1, cfg.D1
    H = H1 if layer == 1 else 1
    GPS = NGRP // NSLICE  # groups per collective slice
    L = f"e{layer}" + sfx
    idxt = con["idxw"]
    maskt = con["maskt"]
    halfrows = cfg.N // 2

    with (
        tc.tile_pool(name=L + "_g", bufs=2) as pg,
        tc.tile_pool(name=L + "_w", bufs=2) as pw,
        tc.tile_pool(name=L + "_e", bufs=2) as pe,
        tc.tile_pool(name=L + "_ps", bufs=2, space="PSUM") as pps,
        tc.tile_pool(name=L + "_ps2", bufs=2, space="PSUM") as pps2,
    ):
        for g in range(NGRP):
            SL, SH = cfg.S_LO[g], cfg.S_HI[g]
            TLO, THI = GRP * SL, GRP * SH
            T = TLO + THI
            goff = cfg.grp_off[g]

            vg = pg.tile([128, T * ROWC], BF16, tag="vg")
            for ci, (sec, t0, k, coloff) in enumerate(cfg.chunks[g]):
                base = TLO if sec else 0
                nidx = k * 128
                nc.gpsimd.dma_gather(
                    out_ap=_ap(vg[:], (base + t0) * ROWC,
                               [[ROWC, k], [1, ROWC]]),
                    in_ap=(vtab[halfrows : 2 * halfrows, :] if sec
                           else vtab[0:halfrows, :]),
                    idxs_ap=idxt[:, coloff : coloff + k * 8],
                    num_idxs=nidx, num_idxs_reg=nidx, elem_size=ROWC,
                    queue_num=int(os.environ.get("KGQ", "0")) and ci % 4,
                )

            # logits lg[p, t*H + h] = asrc[src] + adst[own node p]
            lg = pw.tile([128, T * H], BF16, tag="lg")
            for soff, scnt in ((0, SL), (TLO, SH)):
                if scnt == 0:
                    continue
                if layer == 1:
                    adw_in = _ap(con["adw1"][:], g * GRP * H1,
                                 [[H1, GRP], [0, scnt], [1, H1]])
                else:
                    adw_in = _ap(con["adw2"][:], g * GRP,
                                 [[1, GRP], [0, scnt], [0, 1]])
                nc.vector.tensor_tensor(
                    out=_ap(lg[:], soff * H,
                            [[scnt * H, GRP], [H, scnt], [1, H]]),
                    in0=_ap(vg[:], soff * ROWC + A_OFF,
                            [[scnt * ROWC, GRP], [ROWC, scnt], [1, H]]),
                    in1=adw_in,
                    op=AOP.add,
                )
            lr = pw.tile([128, T * H], BF16, tag="lr")
            nc.vector.scalar_tensor_tensor(
                out=lr[:], in0=lg[:], scalar=NEG_SLOPE, in1=lg[:],
                op0=AOP.mult, op1=AOP.max,
            )
            w_t = pw.tile([128, T * H], BF16, tag="w")
            nc.scalar.activation(out=w_t[:], in_=lr[:], func=ACT.Exp)
            wm = pw.tile([128, T * H], BF16, tag="wm")
            nc.vector.tensor_tensor(
                out=wm[:], in0=w_t[:],
                in1=_ap(maskt[:], goff, [[1, T], [0, H]]),
                op=AOP.mult,
            )

            # weighted values, in place into vg's value cols
            VC = D1 if layer == 1 else 64
            if layer == 1:
                nc.vector.tensor_tensor(
                    out=_ap(vg[:], 0, [[ROWC, T], [H1, C1], [1, H1]]),
                    in0=_ap(vg[:], 0, [[ROWC, T], [H1, C1], [1, H1]]),
                    in1=_ap(wm[:], 0, [[H1, T], [0, C1], [1, H1]]),
                    op=AOP.mult,
                )
            else:
                nc.vector.tensor_tensor(
                    out=_ap(vg[:], 0, [[ROWC, T], [1, 64]]),
                    in0=_ap(vg[:], 0, [[ROWC, T], [1, 64]]),
                    in1=_ap(wm[:], 0, [[1, T], [0, 64]]),
                    op=AOP.mult,
                )

            # per-block reduce over stripes (lo + hi)
            vacc = pe.tile([128, GRP * VC], F32, tag="vacc")
            nc.vector.tensor_reduce(
                out=vacc[:],
                in_=_ap(vg[:], 0, [[SL * ROWC, GRP], [1, VC], [ROWC, SL]]),
                axis=AXL.X, op=AOP.add,
            )
            wsum = pe.tile([128, GRP * H], F32, tag="wsum")
            nc.vector.tensor_reduce(
                out=wsum[:],
                in_=_ap(wm[:], 0, [[SL * H, GRP], [1, H], [H, SL]]),
                axis=AXL.X, op=AOP.add,
            )
            if THI:
                va2 = pe.tile([128, GRP * VC], F32, tag="va2")
                nc.vector.tensor_reduce(
                    out=va2[:],
                    in_=_ap(vg[:], TLO * ROWC,
                            [[SH * ROWC, GRP], [1, VC], [ROWC, SH]]),
                    axis=AXL.X, op=AOP.add,
                )
                nc.vector.tensor_add(out=vacc[:], in0=vacc[:], in1=va2[:])
                ws2 = pe.tile([128, GRP * H], F32, tag="ws2")
                nc.vector.tensor_reduce(
                    out=ws2[:],
                    in_=_ap(wm[:], TLO * H, [[SH * H, GRP], [1, H], [H, SH]]),
                    axis=AXL.X, op=AOP.add,
                )
                nc.vector.tensor_add(out=wsum[:], in0=wsum[:], in1=ws2[:])

            sinv = pe.tile([128, GRP * H], F32, tag="sinv")
            nc.vector.reciprocal(out=sinv[:], in_=wsum[:])

            if layer == 1:
                _l1_epilogue(nc, cfg, g, con, pe, vacc, sinv, t2s)
                # issue slice collectives LAG groups after their data is
                # staged so the Pool sequencer never stalls on t2s waits
                LAG = int(os.environ.get("KLAG", "2"))
                s_ready = (g + 1 - LAG) // GPS - 1
                if (t2 is not None and (g + 1 - LAG) % GPS == 0
                        and s_ready >= 0
                        and os.environ.get("KNOCC", "0") != "1"):
                    _ag_slice(nc, cfg, s_ready, t2s, t2)
            else:
                _l2_epilogue(nc, cfg, g, con, pe, pps, pps2, vacc, sinv, out)

        if (layer == 1 and t2 is not None
                and os.environ.get("KNOCC", "0") != "1"):
            LAG = int(os.environ.get("KLAG", "2"))
            first_tail = max(0, (NGRP - LAG) // GPS)
            for s in range(first_tail, NSLICE):
                _ag_slice(nc, cfg, s, t2s, t2)


def _ag_slice(nc, cfg, s, t2s, t2pair):
    t2tmp_h, t2_h = t2pair
    NSLICE = cfg.NSLICE
    SLN = cfg.NSH // NSLICE
    nc.gpsimd.collective_compute(
        "AllGather", AOP.bypass,
        replica_groups=[list(range(NCORES))],
        ins=[t2s[s * SLN : (s + 1) * SLN, :]],
        outs=[t2tmp_h[s * SLN * NCORES : (s + 1) * SLN * NCORES, :]],
    )
    # local restride: [core, SLN, 65] -> t2 rows c*NSH + s*SLN + i
    nc.sync.dma_start(
        out=bass.AP(
            t2_h[:].tensor, s * SLN * ROWC,
            [[cfg.NSH * ROWC, NCORES], [ROWC, SLN], [1, T2C]],
        ),
        in_=bass.AP(
            t2tmp_h[:].tensor, s * SLN * NCORES * T2C,
            [[SLN * T2C, NCORES], [T2C, SLN], [1, T2C]],
        ),
    )


def _l1_epilogue(nc, cfg, g, con, pe, vacc, sinv, t2s):
    H1, C1, D1 = cfg.H1, cfg.C1, cfg.D1
    # y = vacc/wsum per head (c-major) + b1
    y = pe.tile([128, GRP * D1], F32, tag="y")
    nc.vector.tensor_tensor(
        out=y[:], in0=vacc[:],
        in1=_ap(sinv[:], 0, [[H1, GRP], [0, C1], [1, H1]]),
        op=AOP.mult,
    )
    nc.vector.tensor_tensor(
        out=y[:], in0=y[:],
        in1=_ap(con["b1p"][:], 0, [[0, GRP], [1, D1]]),
        op=AOP.add,
    )
    tmin = pe.tile([128, GRP * D1], F32, tag="tmin")
    nc.vector.tensor_scalar_min(out=tmin[:], in0=y[:], scalar1=0.0)
    e_t = pe.tile([128, GRP * D1], F32, tag="e")
    nc.scalar.activation(out=e_t[:], in_=tmin[:], func=ACT.Exp)
    helu = pe.tile([128, GRP * D1], F32, tag="helu")
    nc.vector.scalar_tensor_tensor(
        out=helu[:], in0=y[:], scalar=0.0, in1=e_t[:],
        op0=AOP.max, op1=AOP.add,
    )
    vq = pe.tile([128, GRP * D1], BF16, tag="vq")
    nc.vector.tensor_scalar_add(out=vq[:], in0=helu[:], scalar1=-1.0)
    # asrc2' = vq . (W2 @ a2s) ; adst2' = vq . (W2 @ a2d)
    asr = pe.tile([128, GRP], F32, tag="asr")
    adt = pe.tile([128, GRP], F32, tag="adt")
    for wsname, dst in (("w2asr", asr), ("w2adr", adt)):
        tmp = pe.tile([128, GRP * D1], BF16, tag="tmp" + wsname)
        nc.vector.tensor_tensor(
            out=tmp[:], in0=vq[:],
            in1=_ap(con[wsname][:], 0, [[0, GRP], [1, D1]]),
            op=AOP.mult,
        )
        nc.vector.tensor_reduce(
            out=dst[:], in_=_ap(tmp[:], 0, [[D1, GRP], [1, D1]]),
            axis=AXL.X, op=AOP.add,
        )
    nc.vector.tensor_copy(
        out=con["adw2"][:, g * GRP : (g + 1) * GRP], in_=adt[:]
    )
    stg = pe.tile([128, GRP * T2C], BF16, tag="stg")
    nc.vector.tensor_copy(
        out=_ap(stg[:], 0, [[T2C, GRP], [1, D1]]),
        in_=_ap(vq[:], 0, [[D1, GRP], [1, D1]]),
    )
    nc.scalar.copy(
        out=_ap(stg[:], D1, [[T2C, GRP], [1, 1]]),
        in_=_ap(asr[:], 0, [[1, GRP], [1, 1]]),
    )
    nc.sync.dma_start(
        out=bass.AP(
            t2s[:].tensor, g * GRP * 128 * T2C,
            [[T2C, 128], [128 * T2C, GRP], [1, T2C]],
        ),
        in_=stg[:].rearrange("p (b w) -> p b w", w=T2C),
    )


def _l2_epilogue(nc, cfg, g, con, pe, pps, pps2, vacc, sinv, out):
    D2 = cfg.D2
    hm = pe.tile([128, GRP * 64], BF16, tag="hm")
    nc.vector.tensor_tensor(
        out=hm[:], in0=vacc[:],
        in1=_ap(sinv[:], 0, [[1, GRP], [0, 64]]),
        op=AOP.mult,
    )
    for b in range(GRP):
        pt = pps.tile([64, 128], BF16, tag="pt")
        nc.tensor.transpose(
            out=pt[:], in_=hm[:, b * 64 : (b + 1) * 64],
            identity=con["identbf"][:],
        )
        hT = pe.tile([64, 128], BF16, tag="hT")
        if b % 2 == 0:
            nc.vector.tensor_copy(out=hT[:], in_=pt[:])
        else:
            nc.scalar.copy(out=hT[:], in_=pt[:])
        ps2 = pps2.tile([128, D2], F32, tag="ps2")
        nc.tensor.matmul(
            out=ps2[:], lhsT=hT[:], rhs=con["w2t"][:], start=True, stop=True
        )
        o_t = pe.tile([128, D2], F32, tag="o")
        nc.vector.tensor_add(out=o_t[:], in0=ps2[:], in1=con["b2r"][:])
        nc.sync.dma_start(
            out=out[(g * GRP + b) * 128 : (g * GRP + b + 1) * 128, :],
            in_=o_t[:],
        )


# ---------------------------------------------------------------------------
# host-side schedule + glue
# ---------------------------------------------------------------------------
def _schedule(src, dst, N):
    """Returns (node_tab [NC, NSH], S_LO [NGRP], S_HI [NGRP], slot arrays).
    Slot arrays are per-edge: core, chunk position info, mask positions."""
    NSH = N // NCORES
    NBLK = NSH // BLK
    NGRP = NBLK // GRP
    # self-loops as ordinary edges
    srcs = np.concatenate([src, np.arange(N, dtype=np.int64)])
    dsts = np.concatenate([dst, np.arange(N, dtype=np.int64)])
    deg = np.bincount(dsts, minlength=N)

    # halves balanced by degree; nlo/nhi depend only on half membership
    order = np.argsort(-deg, kind="stable")
    half_of = np.empty(N, dtype=np.int64)
    half_of[order] = np.arange(N) % 2
    is_lo = half_of == 0

    hi_flag = (~is_lo[srcs]).astype(np.int64)
    nlo = np.bincount(dsts[hi_flag == 0], minlength=N)
    nhi = deg - nlo

    # per half: ONE global lex sort by (nlo, nhi); consecutive runs of
    # 4*GRP*BLK nodes form group g across the half's 4 cores (dealt
    # round-robin), so every core's group-g block set is statistically
    # identical -> the global per-group section maxima stay tight.
    HC = NCORES // 2
    RUN = HC * GRP * BLK
    NGRP = NSH // (GRP * BLK)
    node_tab = np.empty((NCORES, NSH), dtype=np.int64)
    for h in range(2):
        nodes = np.where(half_of == h)[0]
        # quantile grid: 4 nlo bands (4 runs each), nhi-sorted within band,
        # so each run is tight in BOTH nlo and nhi
        nl = nlo[nodes]
        o1 = np.argsort(nl, kind="stable")
        qlo = np.empty(len(nodes), dtype=np.int64)
        qlo[o1] = np.arange(len(nodes)) // (4 * RUN)
        o = np.lexsort((nhi[nodes], qlo))
        runs = nodes[o].reshape(NGRP, GRP * BLK, HC)
        node_tab[h * HC : (h + 1) * HC] = runs.transpose(2, 0, 1).reshape(
            HC, NGRP * GRP * BLK)
    phys = np.empty(N, dtype=np.int64)
    phys[node_tab.reshape(-1)] = np.arange(N)

    # per-group section heights (global max over cores)
    nlo_b = nlo[node_tab].reshape(NCORES, NGRP, GRP * BLK)
    nhi_b = nhi[node_tab].reshape(NCORES, NGRP, GRP * BLK)
    S_LO = np.maximum(nlo_b.max(axis=(0, 2)), 1)
    S_HI = np.maximum(nhi_b.max(axis=(0, 2)), 1)

    # per-edge rank within (dst, sec)
    eo = np.lexsort((hi_flag, dsts))
    sd = dsts[eo]
    sh = hi_flag[eo]
    ss = srcs[eo]
    key_change = np.ones(len(sd), dtype=bool)
    key_change[1:] = (sd[1:] != sd[:-1]) | (sh[1:] != sh[:-1])
    gidx = np.flatnonzero(key_change)
    grp_id = np.cumsum(key_change) - 1
    rank = np.arange(len(sd)) - gidx[grp_id]

    return (node_tab, phys, S_LO, S_HI, sd, sh, ss, rank)


def prepare(x, seq, edges, W1, att_src1, att_dst1, b1, W2, att_src2,
            att_dst2, b2, nslice=4):
    import ml_dtypes

    bf = ml_dtypes.bfloat16
    nb, ncn, d = x.shape
    N = nb * ncn
    H1, C1 = att_src1.shape
    D1 = H1 * C1
    D2 = W2.shape[1]
    NSH = N // NCORES
    NBLK = NSH // BLK

    xf = (np.asarray(x, np.float32).reshape(N, d)
          * np.asarray(seq, np.float32).reshape(N, 1))
    src = np.asarray(edges[0], np.int64)
    dst = np.asarray(edges[1], np.int64)
    node_tab, phys, S_LO, S_HI, sd, sh, ss, rank = _schedule(src, dst, N)
    cfg = Cfg(N, d, H1, C1, D2, S_LO, S_HI, nslice)

    # ---- weights / consts -------------------------------------------------
    new2old = np.empty(D1, dtype=np.int64)
    for c in range(C1):
        for h in range(H1):
            new2old[c * H1 + h] = h * C1 + c
    w1 = np.asarray(W1, np.float32)
    wsrc = np.einsum("khc,hc->kh", w1.reshape(d, H1, C1),
                     np.asarray(att_src1, np.float32))
    wdst = np.einsum("khc,hc->kh", w1.reshape(d, H1, C1),
                     np.asarray(att_dst1, np.float32))
    wpack1 = np.concatenate([w1[:, new2old], wsrc], axis=1).astype(bf)

    w2p = np.asarray(W2, np.float32)[new2old, :]
    a2s = np.asarray(att_src2, np.float32).reshape(-1)
    a2d = np.asarray(att_dst2, np.float32).reshape(-1)
    w2as = w2p @ a2s    # [D1] c-major
    w2ad = w2p @ a2d
    w2asr = np.tile(w2as[None, :], (128, 1)).astype(bf)
    w2adr = np.tile(w2ad[None, :], (128, 1)).astype(bf)
    b1p = np.tile(np.asarray(b1, np.float32)[new2old][None, :],
                  (128, 1)).astype(np.float32)
    b2r = np.tile(np.asarray(b2, np.float32)[None, :], (128, 1)).astype(
        np.float32)
    identbf = np.eye(128, dtype=np.float32).astype(bf)

    # xt: global phys order (same for every core)
    xt = np.ascontiguousarray(xf[node_tab.reshape(-1)].T).astype(bf)

    # adw1: a_dst1 of own nodes, [128, NBLK*H1] per core
    ad_all = (xf @ wdst).astype(np.float32)          # [N, H1]
    adw1_all = ad_all[node_tab].reshape(NCORES, NBLK, BLK, H1)
    adw1_all = adw1_all.transpose(0, 2, 1, 3).reshape(
        NCORES, BLK, NBLK * H1).astype(bf)

    # ---- slot placement (vectorized) -------------------------------------
    # per-edge destination placement
    pd = phys[sd]                       # phys row of dst
    e_core = pd // NSH
    loc = pd % NSH
    e_blk = loc // BLK                  # block in core
    e_p = loc % BLK                     # partition
    e_g = e_blk // GRP
    e_b = e_blk % GRP
    slo_g = S_LO[e_g]
    shi_g = S_HI[e_g]
    # stripe within the group's section-stripe space
    ts = np.where(sh == 0, e_b * slo_g + rank, e_b * shi_g + rank)
    # chunk col offsets: build lookup per (group, sec, chunk_index)
    chcol = {}
    for g in range(cfg.NGRP):
        for (sec, t0, k, col) in cfg.chunks[g]:
            chcol[(g, sec, t0 // CHST)] = col
    max_ch = max(
        max((t0 // CHST) for (sec, t0, k, col) in cfg.chunks[g]
            if True) for g in range(cfg.NGRP)) + 1
    col_lut = np.full((cfg.NGRP, 2, max_ch + 1), -1, dtype=np.int64)
    for (g, sec, ci), col in chcol.items():
        col_lut[g, sec, ci] = col
    ci = ts // CHST
    tin = ts % CHST
    pos = tin * 128 + e_p               # position within chunk
    basecol = col_lut[e_g, sh, ci]
    assert (basecol >= 0).all()
    idx_col = basecol + pos // 16
    idx_row = pos % 16
    idx_val = np.where(sh == 0, phys[ss], phys[ss] - N // 2).astype(np.int16)
    # mask position: group stripe offset + section offset + ts
    grp_off = np.asarray(cfg.grp_off, dtype=np.int64)
    sec_off = np.where(sh == 0, 0, GRP * slo_g)
    mask_col = grp_off[e_g] + sec_off + ts

    idxw = np.zeros((NCORES, 128, cfg.idx_cols), dtype=np.int16)
    maskv = np.zeros((NCORES, 128, cfg.tot_stripes), dtype=np.float32)
    for c in range(NCORES):
        m = e_core == c
        # idx wrapped layout: row pos%16, replicated over 8 row-groups
        rows = idx_row[m]
        cols = idx_col[m]
        vals = idx_val[m]
        for repk in range(8):
            idxw[c][rows + 16 * repk, cols] = vals
        maskv[c][e_p[m], mask_col[m]] = 1.0
    maskv = maskv.astype(bf)

    in_maps = []
    for c in range(NCORES):
        in_maps.append({
            "xt": xt, "wpack1": wpack1, "w2t": w2p.astype(bf),
            "w2asr": w2asr, "w2adr": w2adr, "b1p": b1p, "b2r": b2r,
            "identbf": identbf, "idxw": idxw[c], "maskt": maskv[c],
            "adw1": adw1_all[c],
        })
    return cfg, node_tab, in_maps


_CACHE = {}
LAST_RESULT = None


def kernel(**inputs) -> np.ndarray:
    from concourse.bass_utils import run_bass_kernel_spmd

    global LAST_RESULT
    x = np.asarray(inputs["x"])
    nb, ncn, d = x.shape
    nslice = int(os.environ.get("KNSLICE", "8"))
    cfg, node_tab, in_maps = prepare(**{k: inputs[k] for k in (
        "x", "seq", "edges", "W1", "att_src1", "att_dst1", "b1",
        "W2", "att_src2", "att_dst2", "b2")}, nslice=nslice)

    phases = os.environ.get("KPHASES", "abc")
    key = (cfg.N, cfg.D, cfg.H1, cfg.C1, cfg.D2, tuple(cfg.S_LO),
           tuple(cfg.S_HI), cfg.NSLICE, phases)
    if key not in _CACHE:
        _CACHE.clear()
        _CACHE[key] = build_program(cfg, phases=phases)
    nc = _CACHE[key]

    res = run_bass_kernel_spmd(nc, in_maps, core_ids=list(range(NCORES)),
                               trace=False)
    LAST_RESULT = res
    shards = np.concatenate(
        [res.results[c]["out"] for c in range(NCORES)], axis=0)
    full = np.empty((cfg.N, cfg.D2), dtype=np.float32)
    full[node_tab.reshape(-1)] = shards
    return full.reshape(nb, ncn, d).astype(np.float32)
